# revision 10
# baseline (speedup 1.0000x reference)
"""Edge-parallel GNN message-passing MLP on 8 TRN2 NeuronCores.

Computation (per edge e): out[e] = relu(concat(x[row[e]], edge_attr[e]) @ W1 + b1) @ W2 + b2

Sharding: edges are SORTED BY ROW on the host, then split evenly across the 8
cores (edge-parallel). Sorting keeps each core's rows in a contiguous band of
~6250 nodes (gather indices fit int16 against the band base) and makes
adjacent edges usually share a row.

Host-side stream construction (per core):
  - edges are packed into PAIRS that share a row (~3% dummy duplicate edges
    inserted at odd-length run boundaries), so one 256 B gather token serves
    two edges: halves gather descriptors and gather HBM traffic
  - x rows are zero-padded to 256 B fp16 tokens [64 feats | 64 zeros]
  - edge_attr is pre-transposed to [64, E] fp16 in stream order
  - the within-tile edge order is chosen so the output DMA writes 4 KiB
    contiguous DRAM per partition (8 consecutive rows), 8x fewer descriptors

Device per 2048-edge tile:
  - dma_gather (non-transpose, 256 B tokens, 1024 pair indices) on SWDGE
    queue t%4 — 4 Q7 core-pairs generate descriptors concurrently
  - PE "transpose" of each 128-pair block against a [128, 256] duplication
    matrix D (D[i,2i]=D[i,2i+1]=1) yields feature-major fp16 x features with
    each pair column already duplicated per edge
  - DVE copies them into the feats tile; edge_attr DMAs into partitions
    64:127; L1 fp16 matmul + fused relu+b1 on ScalarE; L2 per-block matmuls
    (FWL fp16 weight loads); DVE adds b2 -> fp32; 4 KiB-per-partition output
    DMA; host scatters rows back to original edge order.
"""

from contextlib import ExitStack

import numpy as np

import concourse.bacc as bacc_mod
import concourse.bass as bass
import concourse.mybir as mybir
import concourse.tile as tile
from concourse.bass_utils import run_bass_kernel_spmd

N_CORES = 8
N_NODES = 50000
N_EDGES = 800000
F_IN = 64
HIDDEN = 128
F_OUT = 128

E_REAL = N_EDGES // N_CORES  # 100000 edges per core
TILE_E = 2048                # edges per pipeline tile
GRP = 4                      # edges per gather token (same-row groups)
GRPS_T = TILE_E // GRP       # 512 gather tokens per tile
NT = 54                      # tiles per core (room for ~10% grouping dummies)
EPC = NT * TILE_E            # padded edge-stream length per core
HALF = 1024                  # edges per PSUM-stage half-tile
BAND = 8192                  # max x rows referenced per core (sorted band)

F32 = mybir.dt.float32
F16 = mybir.dt.float16
I16 = mybir.dt.int16

RELU = mybir.ActivationFunctionType.Relu
ADD = mybir.AluOpType.add


def build_program(nt: int = NT):
    epc = nt * TILE_E
    nc = bacc_mod.Bacc("TRN2", num_swdge_queues=4)

    xtok_d = nc.declare_dram_parameter("xtok", [BAND, 2 * F_IN], F16, isOutput=False)
    # pair indices (row - band_base), [16, n/16]-wrapped per tile, replicated
    # x8 across partitions so every SWDGE queue's Q7 pair sees its copy
    idx_d = nc.declare_dram_parameter("idx", [128, nt * (GRPS_T // 16)], I16, isOutput=False)
    eat_d = nc.declare_dram_parameter("eat", [F_IN, epc], F16, isOutput=False)
    ident_d = nc.declare_dram_parameter("ident", [128, 128], F16, isOutput=False)
    w1_d = nc.declare_dram_parameter("w1", [2 * F_IN, HIDDEN], F16, isOutput=False)
    w2_d = nc.declare_dram_parameter("w2", [HIDDEN, F_OUT], F16, isOutput=False)
    b1_d = nc.declare_dram_parameter("b1c", [HIDDEN, 1], F32, isOutput=False)
    b2_d = nc.declare_dram_parameter("b2c", [F_OUT, 1], F32, isOutput=False)
    # feature-major output: column q = stream edge q; host transposes back
    out_d = nc.declare_dram_parameter("out", [F_OUT, epc], F32, isOutput=True)

    with tile.TileContext(nc) as tc, ExitStack() as ctx:
        const = ctx.enter_context(tc.tile_pool(name="const", bufs=1))
        xg_p = ctx.enter_context(tc.tile_pool(name="xg", bufs=6))
        feats_p = ctx.enter_context(tc.tile_pool(name="feats", bufs=4))
        h1sb_p = ctx.enter_context(tc.tile_pool(name="h1sb", bufs=3))
        outsb_p = ctx.enter_context(tc.tile_pool(name="outsb", bufs=3))
        xpt_p = ctx.enter_context(tc.tile_pool(name="xpt", bufs=2, space="PSUM"))
        h1ps_p = ctx.enter_context(tc.tile_pool(name="h1ps", bufs=2, space="PSUM"))
        outps_p = ctx.enter_context(tc.tile_pool(name="outps", bufs=2, space="PSUM"))

        # ---- constants (loaded once) ----
        w1_t = const.tile([128, HIDDEN], F16, tag="w1")
        nc.sync.dma_start(out=w1_t, in_=w1_d[:, :])
        w2_t = const.tile([128, F_OUT], F16, tag="w2")
        nc.sync.dma_start(out=w2_t, in_=w2_d[:, :])
        ident_t = const.tile([128, 128], F16, tag="ident")
        nc.sync.dma_start(out=ident_t, in_=ident_d[:, :])
        b1_t = const.tile([128, 1], F32, tag="b1")
        nc.sync.dma_start(out=b1_t, in_=b1_d[:, :])
        b2_t = const.tile([128, 1], F32, tag="b2")
        nc.sync.dma_start(out=b2_t, in_=b2_d[:, :])
        idx_t = const.tile([128, nt * (GRPS_T // 16)], I16, tag="idx")
        nc.sync.dma_start(out=idx_t, in_=idx_d[:, :])

        S = GRPS_T // 16
        for t in range(nt):
            # ---- gather group tokens, group-major [grp%128, grp//128, elem] ----
            xg = xg_p.tile([128, GRPS_T // 128, 2 * F_IN], F16, tag="xg")
            nc.gpsimd.dma_gather(
                xg[:, :, :],
                xtok_d[:, :],
                idx_t[:, t * S : (t + 1) * S],
                GRPS_T,
                GRPS_T,
                2 * F_IN,
                transpose=False,
                single_packet=False,
                queue_num=t % 4,
            )

            # ---- feature-major via PE transposes: xpT[f, grp] ----
            NB = GRPS_T // 128  # 4 transpose blocks
            xpt = xpt_p.tile([F_IN, NB, 128], F16, tag="xpt", space="PSUM")
            for b in range(NB):
                nc.tensor.transpose(
                    out=xpt[:, b, :],
                    in_=xg[:, b, 0:F_IN],
                    identity=ident_t,
                )
            # copy group columns duplicated per edge (stride-0 dup dim), split
            # across DVE and ScalarE: feats col 512*b + 4*k + d = grp 128*b + k
            feats = feats_p.tile([128, TILE_E], F16, tag="feats")
            lo = xpt[:, 0 : NB // 2, :]
            dup_lo = bass.AP(lo.tensor, lo.offset, [*lo.ap, [0, GRP]])
            hi = xpt[:, NB // 2 :, :]
            dup_hi = bass.AP(hi.tensor, hi.offset, [*hi.ap, [0, GRP]])
            nc.vector.tensor_copy(
                out=feats[0:F_IN, 0 : TILE_E // 2].rearrange(
                    "f (b e d) -> f b e d", b=NB // 2, e=128, d=GRP
                ),
                in_=dup_lo,
            )
            nc.scalar.copy(
                out=feats[0:F_IN, TILE_E // 2 : TILE_E].rearrange(
                    "f (b e d) -> f b e d", b=NB // 2, e=128, d=GRP
                ),
                in_=dup_hi,
            )
            nc.sync.dma_start(
                out=feats[F_IN : 2 * F_IN, :],
                in_=eat_d[:, t * TILE_E : (t + 1) * TILE_E],
            )

            outsb = outsb_p.tile([128, TILE_E], F32, tag="outsb")
            for h in range(TILE_E // HALF):
                fh = feats[:, h * HALF : (h + 1) * HALF]

                # ---- layer 1: h1T[H, 1024] = W1.T @ feats ----
                h1ps = h1ps_p.tile([128, HALF], F32, tag="h1ps", space="PSUM")
                for q in range(HALF // 512):
                    nc.tensor.matmul(
                        out=h1ps[:, q * 512 : (q + 1) * 512],
                        lhsT=w1_t,
                        rhs=fh[:, q * 512 : (q + 1) * 512],
                        start=True,
                        stop=True,
                    )
                h1sb = h1sb_p.tile([128, HALF], F16, tag="h1sb")
                nc.scalar.activation(
                    out=h1sb, in_=h1ps, func=RELU, bias=b1_t, scale=1.0
                )

                # ---- layer 2 + bias, feature-major (W2 stationary) ----
                for half2 in range(2):
                    outps = outps_p.tile([128, 512], F32, tag="outps", space="PSUM")
                    nc.tensor.matmul(
                        out=outps,
                        lhsT=w2_t,
                        rhs=h1sb[:, half2 * 512 : (half2 + 1) * 512],
                        start=True,
                        stop=True,
                    )
                    nc.vector.tensor_tensor(
                        out=outsb[:, h * HALF + half2 * 512 : h * HALF + (half2 + 1) * 512],
                        in0=outps,
                        in1=b2_t.to_broadcast([128, 512]),
                        op=ADD,
                    )
            nc.sync.dma_start(
                out=out_d[:, t * TILE_E : (t + 1) * TILE_E],
                in_=outsb,
            )

    nc.compile()
    return nc


_PROG = None


def _get_prog():
    global _PROG
    if _PROG is None:
        _PROG = build_program(NT)
    return _PROG


def _group_stream(rows_c):
    """Pack sorted rows into same-row groups of GRP edges, duplicating the
    last edge of short runs. Returns local edge indices, one per slot."""
    n = len(rows_c)
    change = np.flatnonzero(np.diff(rows_c)) + 1
    run_starts = np.concatenate([[0], change])
    run_lens = np.diff(np.concatenate([run_starts, [n]]))
    ngrp = (run_lens + GRP - 1) // GRP
    total = int(ngrp.sum())
    grp_run = np.repeat(np.arange(len(run_lens)), ngrp)
    first = np.cumsum(ngrp) - ngrp
    grp_off = np.arange(total) - first[grp_run]
    base = run_starts[grp_run] + GRP * grp_off
    last = run_starts[grp_run] + run_lens[grp_run] - 1
    stream = np.empty((total, GRP), dtype=np.int64)
    for j in range(GRP):
        stream[:, j] = np.minimum(base + j, last)
    return stream.reshape(-1)


def _prepare_in_maps(x, edge_index, edge_attr, W1, b1, W2, b2):
    row = np.ascontiguousarray(np.asarray(edge_index)[0]).astype(np.int64)
    order = np.argsort(row, kind="stable")
    row_s = row[order]
    ea_s = np.asarray(edge_attr, dtype=np.float32)[order]
    x16 = np.asarray(x, dtype=np.float32).astype(np.float16)
    w1_16 = np.ascontiguousarray(np.asarray(W1, dtype=np.float32).astype(np.float16))
    w2_16 = np.ascontiguousarray(np.asarray(W2, dtype=np.float32).astype(np.float16))
    b1c = np.ascontiguousarray(np.asarray(b1, dtype=np.float32).reshape(HIDDEN, 1))
    b2v = np.ascontiguousarray(np.asarray(b2, dtype=np.float32).reshape(F_OUT, 1))

    ident = np.eye(128, dtype=np.float16)

    in_maps = []
    streams = []
    for c in range(N_CORES):
        sl = slice(c * E_REAL, (c + 1) * E_REAL)
        rows_c = row_s[sl]
        r0 = int(rows_c[0])
        band_n = int(rows_c[-1]) - r0 + 1
        assert band_n <= BAND, (c, band_n)

        stream = _group_stream(rows_c)
        assert len(stream) <= EPC, (c, len(stream))
        stream_pad = np.zeros(EPC, dtype=np.int64)
        stream_pad[: len(stream)] = stream
        streams.append((stream, len(stream)))

        grp_rows = (rows_c[stream_pad[0::GRP]] - r0).astype(np.int16)  # [EPC//GRP]
        idx_t = np.ascontiguousarray(
            np.tile(
                grp_rows.reshape(NT, GRPS_T // 16, 16).transpose(0, 2, 1), (1, 8, 1)
            ).transpose(1, 0, 2)
        ).reshape(128, NT * (GRPS_T // 16))

        xb = np.zeros((BAND, 2 * F_IN), dtype=np.float16)
        nb = min(BAND, N_NODES - r0)
        xb[:nb, :F_IN] = x16[r0 : r0 + nb]

        eat = np.ascontiguousarray(ea_s[sl][stream_pad].astype(np.float16).T)

        in_maps.append(
            {
                "xtok": xb,
                "idx": idx_t,
                "eat": eat,
                "ident": ident,
                "w1": w1_16,
                "w2": w2_16,
                "b1c": b1c,
                "b2c": b2v,
            }
        )
    return in_maps, order, streams


def run_spmd(inputs: dict, trace: bool = False, **spmd_kwargs):
    """Run the kernel on all 8 cores. Returns (output, BassKernelResults)."""
    in_maps, order, streams = _prepare_in_maps(
        inputs["x"], inputs["edge_index"], inputs["edge_attr"],
        inputs["W1"], inputs["b1"], inputs["W2"], inputs["b2"],
    )
    nc = _get_prog()
    bres = run_bass_kernel_spmd(
        nc, in_maps, list(range(N_CORES)), trace=trace, **spmd_kwargs
    )
    res = bres.results

    out = np.empty((N_EDGES, F_OUT), dtype=np.float32)
    for c in range(N_CORES):
        stream, slen = streams[c]
        core_out = res[c]["out"]  # [128, EPC] feature-major, col q = stream edge q
        sl_ids = order[c * E_REAL : (c + 1) * E_REAL]
        out[sl_ids[stream]] = core_out[:, :slen].T
    return out, bres


def kernel(x, edge_index, edge_attr, u, batch, W1, b1, W2, b2):
    out, _ = run_spmd(
        {
            "x": x, "edge_index": edge_index, "edge_attr": edge_attr,
            "W1": W1, "b1": b1, "W2": W2, "b2": b2,
        }
    )
    return out


# revision 11
# speedup vs baseline: 1.1949x; 1.1949x over previous
"""Edge-parallel GNN message-passing MLP on 8 TRN2 NeuronCores.

Computation (per edge e): out[e] = relu(concat(x[row[e]], edge_attr[e]) @ W1 + b1) @ W2 + b2

Sharding: edges are SORTED BY ROW on the host, then split evenly across the 8
cores (edge-parallel). Sorting keeps each core's rows in a contiguous band of
~6250 nodes (gather indices fit int16 against the band base) and makes
adjacent edges usually share a row.

Host-side stream construction (per core):
  - edges are packed into PAIRS that share a row (~3% dummy duplicate edges
    inserted at odd-length run boundaries), so one 256 B gather token serves
    two edges: halves gather descriptors and gather HBM traffic
  - x rows are zero-padded to 256 B fp16 tokens [64 feats | 64 zeros]
  - edge_attr is pre-transposed to [64, E] fp16 in stream order
  - the within-tile edge order is chosen so the output DMA writes 4 KiB
    contiguous DRAM per partition (8 consecutive rows), 8x fewer descriptors

Device per 2048-edge tile:
  - dma_gather (non-transpose, 256 B tokens, 1024 pair indices) on SWDGE
    queue t%4 — 4 Q7 core-pairs generate descriptors concurrently
  - PE "transpose" of each 128-pair block against a [128, 256] duplication
    matrix D (D[i,2i]=D[i,2i+1]=1) yields feature-major fp16 x features with
    each pair column already duplicated per edge
  - DVE copies them into the feats tile; edge_attr DMAs into partitions
    64:127; L1 fp16 matmul + fused relu+b1 on ScalarE; L2 per-block matmuls
    (FWL fp16 weight loads); DVE adds b2 -> fp32; 4 KiB-per-partition output
    DMA; host scatters rows back to original edge order.
"""

from contextlib import ExitStack

import numpy as np

import concourse.bacc as bacc_mod
import concourse.bass as bass
import concourse.mybir as mybir
import concourse.tile as tile
from concourse.bass_utils import run_bass_kernel_spmd

N_CORES = 8
N_NODES = 50000
N_EDGES = 800000
F_IN = 64
HIDDEN = 128
F_OUT = 128

E_REAL = N_EDGES // N_CORES  # 100000 edges per core
TILE_E = 2048                # edges per pipeline tile
GRP = 4                      # edges per gather token (same-row groups)
GRPS_T = TILE_E // GRP       # 512 gather tokens per tile
NT = 54                      # tiles per core (room for ~10% grouping dummies)
EPC = NT * TILE_E            # padded edge-stream length per core
HALF = 1024                  # edges per PSUM-stage half-tile
BAND = 8192                  # max x rows referenced per core (sorted band)

F32 = mybir.dt.float32
F16 = mybir.dt.float16
I16 = mybir.dt.int16

RELU = mybir.ActivationFunctionType.Relu
ADD = mybir.AluOpType.add


def build_program(nt: int = NT):
    epc = nt * TILE_E
    nc = bacc_mod.Bacc("TRN2", num_swdge_queues=4)

    xtok_d = nc.declare_dram_parameter("xtok", [BAND, 2 * F_IN], F16, isOutput=False)
    # pair indices (row - band_base), [16, n/16]-wrapped per tile, replicated
    # x8 across partitions so every SWDGE queue's Q7 pair sees its copy
    idx_d = nc.declare_dram_parameter("idx", [128, nt * (GRPS_T // 16)], I16, isOutput=False)
    eat_d = nc.declare_dram_parameter("eat", [F_IN, epc], F16, isOutput=False)
    ident_d = nc.declare_dram_parameter("ident", [128, 128], F16, isOutput=False)
    w1_d = nc.declare_dram_parameter("w1", [2 * F_IN, HIDDEN], F16, isOutput=False)
    w2_d = nc.declare_dram_parameter("w2", [HIDDEN, F_OUT], F16, isOutput=False)
    b1_d = nc.declare_dram_parameter("b1c", [HIDDEN, 1], F32, isOutput=False)
    b2_d = nc.declare_dram_parameter("b2c", [F_OUT, 1], F32, isOutput=False)
    # feature-major fp16 output: column q = stream edge q; host transposes
    # back and upconverts (fp16 rounding adds ~3e-4 RMS, well within budget)
    out_d = nc.declare_dram_parameter("out", [F_OUT, epc], F16, isOutput=True)

    with tile.TileContext(nc) as tc, ExitStack() as ctx:
        const = ctx.enter_context(tc.tile_pool(name="const", bufs=1))
        xg_p = ctx.enter_context(tc.tile_pool(name="xg", bufs=6))
        feats_p = ctx.enter_context(tc.tile_pool(name="feats", bufs=4))
        h1sb_p = ctx.enter_context(tc.tile_pool(name="h1sb", bufs=3))
        outsb_p = ctx.enter_context(tc.tile_pool(name="outsb", bufs=3))
        xpt_p = ctx.enter_context(tc.tile_pool(name="xpt", bufs=2, space="PSUM"))
        h1ps_p = ctx.enter_context(tc.tile_pool(name="h1ps", bufs=2, space="PSUM"))
        outps_p = ctx.enter_context(tc.tile_pool(name="outps", bufs=2, space="PSUM"))

        # ---- constants (loaded once) ----
        w1_t = const.tile([128, HIDDEN], F16, tag="w1")
        nc.sync.dma_start(out=w1_t, in_=w1_d[:, :])
        w2_t = const.tile([128, F_OUT], F16, tag="w2")
        nc.sync.dma_start(out=w2_t, in_=w2_d[:, :])
        ident_t = const.tile([128, 128], F16, tag="ident")
        nc.sync.dma_start(out=ident_t, in_=ident_d[:, :])
        b1_t = const.tile([128, 1], F32, tag="b1")
        nc.sync.dma_start(out=b1_t, in_=b1_d[:, :])
        b2_t = const.tile([128, 1], F32, tag="b2")
        nc.sync.dma_start(out=b2_t, in_=b2_d[:, :])
        idx_t = const.tile([128, nt * (GRPS_T // 16)], I16, tag="idx")
        nc.sync.dma_start(out=idx_t, in_=idx_d[:, :])

        S = GRPS_T // 16
        for t in range(nt):
            # ---- gather group tokens, group-major [grp%128, grp//128, elem] ----
            xg = xg_p.tile([128, GRPS_T // 128, 2 * F_IN], F16, tag="xg")
            nc.gpsimd.dma_gather(
                xg[:, :, :],
                xtok_d[:, :],
                idx_t[:, t * S : (t + 1) * S],
                GRPS_T,
                GRPS_T,
                2 * F_IN,
                transpose=False,
                single_packet=False,
                queue_num=t % 4,
            )

            # ---- feature-major via PE transposes: xpT[f, grp] ----
            NB = GRPS_T // 128  # 4 transpose blocks
            xpt = xpt_p.tile([F_IN, NB, 128], F16, tag="xpt", space="PSUM")
            for b in range(NB):
                nc.tensor.transpose(
                    out=xpt[:, b, :],
                    in_=xg[:, b, 0:F_IN],
                    identity=ident_t,
                )
            # copy group columns duplicated per edge (stride-0 dup dim), split
            # across DVE and ScalarE: feats col 512*b + 4*k + d = grp 128*b + k
            feats = feats_p.tile([128, TILE_E], F16, tag="feats")
            lo = xpt[:, 0 : NB // 2, :]
            dup_lo = bass.AP(lo.tensor, lo.offset, [*lo.ap, [0, GRP]])
            hi = xpt[:, NB // 2 :, :]
            dup_hi = bass.AP(hi.tensor, hi.offset, [*hi.ap, [0, GRP]])
            nc.vector.tensor_copy(
                out=feats[0:F_IN, 0 : TILE_E // 2].rearrange(
                    "f (b e d) -> f b e d", b=NB // 2, e=128, d=GRP
                ),
                in_=dup_lo,
            )
            nc.scalar.copy(
                out=feats[0:F_IN, TILE_E // 2 : TILE_E].rearrange(
                    "f (b e d) -> f b e d", b=NB // 2, e=128, d=GRP
                ),
                in_=dup_hi,
            )
            nc.sync.dma_start(
                out=feats[F_IN : 2 * F_IN, :],
                in_=eat_d[:, t * TILE_E : (t + 1) * TILE_E],
            )

            outsb = outsb_p.tile([128, TILE_E], F16, tag="outsb")
            for h in range(TILE_E // HALF):
                fh = feats[:, h * HALF : (h + 1) * HALF]

                # ---- layer 1: h1T[H, 1024] = W1.T @ feats ----
                h1ps = h1ps_p.tile([128, HALF], F32, tag="h1ps", space="PSUM")
                for q in range(HALF // 512):
                    nc.tensor.matmul(
                        out=h1ps[:, q * 512 : (q + 1) * 512],
                        lhsT=w1_t,
                        rhs=fh[:, q * 512 : (q + 1) * 512],
                        start=True,
                        stop=True,
                    )
                h1sb = h1sb_p.tile([128, HALF], F16, tag="h1sb")
                nc.scalar.activation(
                    out=h1sb, in_=h1ps, func=RELU, bias=b1_t, scale=1.0
                )

                # ---- layer 2 + bias, feature-major (W2 stationary) ----
                for half2 in range(2):
                    outps = outps_p.tile([128, 512], F32, tag="outps", space="PSUM")
                    nc.tensor.matmul(
                        out=outps,
                        lhsT=w2_t,
                        rhs=h1sb[:, half2 * 512 : (half2 + 1) * 512],
                        start=True,
                        stop=True,
                    )
                    nc.vector.tensor_tensor(
                        out=outsb[:, h * HALF + half2 * 512 : h * HALF + (half2 + 1) * 512],
                        in0=outps,
                        in1=b2_t.to_broadcast([128, 512]),
                        op=ADD,
                    )
            nc.sync.dma_start(
                out=out_d[:, t * TILE_E : (t + 1) * TILE_E],
                in_=outsb,
            )

    nc.compile()
    return nc


_PROG = None


def _get_prog():
    global _PROG
    if _PROG is None:
        _PROG = build_program(NT)
    return _PROG


def _group_stream(rows_c):
    """Pack sorted rows into same-row groups of GRP edges, duplicating the
    last edge of short runs. Returns local edge indices, one per slot."""
    n = len(rows_c)
    change = np.flatnonzero(np.diff(rows_c)) + 1
    run_starts = np.concatenate([[0], change])
    run_lens = np.diff(np.concatenate([run_starts, [n]]))
    ngrp = (run_lens + GRP - 1) // GRP
    total = int(ngrp.sum())
    grp_run = np.repeat(np.arange(len(run_lens)), ngrp)
    first = np.cumsum(ngrp) - ngrp
    grp_off = np.arange(total) - first[grp_run]
    base = run_starts[grp_run] + GRP * grp_off
    last = run_starts[grp_run] + run_lens[grp_run] - 1
    stream = np.empty((total, GRP), dtype=np.int64)
    for j in range(GRP):
        stream[:, j] = np.minimum(base + j, last)
    return stream.reshape(-1)


def _prepare_in_maps(x, edge_index, edge_attr, W1, b1, W2, b2):
    row = np.ascontiguousarray(np.asarray(edge_index)[0]).astype(np.int64)
    order = np.argsort(row, kind="stable")
    row_s = row[order]
    ea_s = np.asarray(edge_attr, dtype=np.float32)[order]
    x16 = np.asarray(x, dtype=np.float32).astype(np.float16)
    w1_16 = np.ascontiguousarray(np.asarray(W1, dtype=np.float32).astype(np.float16))
    w2_16 = np.ascontiguousarray(np.asarray(W2, dtype=np.float32).astype(np.float16))
    b1c = np.ascontiguousarray(np.asarray(b1, dtype=np.float32).reshape(HIDDEN, 1))
    b2v = np.ascontiguousarray(np.asarray(b2, dtype=np.float32).reshape(F_OUT, 1))

    ident = np.eye(128, dtype=np.float16)

    in_maps = []
    streams = []
    for c in range(N_CORES):
        sl = slice(c * E_REAL, (c + 1) * E_REAL)
        rows_c = row_s[sl]
        r0 = int(rows_c[0])
        band_n = int(rows_c[-1]) - r0 + 1
        assert band_n <= BAND, (c, band_n)

        stream = _group_stream(rows_c)
        assert len(stream) <= EPC, (c, len(stream))
        stream_pad = np.zeros(EPC, dtype=np.int64)
        stream_pad[: len(stream)] = stream
        streams.append((stream, len(stream)))

        grp_rows = (rows_c[stream_pad[0::GRP]] - r0).astype(np.int16)  # [EPC//GRP]
        idx_t = np.ascontiguousarray(
            np.tile(
                grp_rows.reshape(NT, GRPS_T // 16, 16).transpose(0, 2, 1), (1, 8, 1)
            ).transpose(1, 0, 2)
        ).reshape(128, NT * (GRPS_T // 16))

        xb = np.zeros((BAND, 2 * F_IN), dtype=np.float16)
        nb = min(BAND, N_NODES - r0)
        xb[:nb, :F_IN] = x16[r0 : r0 + nb]

        eat = np.ascontiguousarray(ea_s[sl][stream_pad].astype(np.float16).T)

        in_maps.append(
            {
                "xtok": xb,
                "idx": idx_t,
                "eat": eat,
                "ident": ident,
                "w1": w1_16,
                "w2": w2_16,
                "b1c": b1c,
                "b2c": b2v,
            }
        )
    return in_maps, order, streams


def run_spmd(inputs: dict, trace: bool = False, **spmd_kwargs):
    """Run the kernel on all 8 cores. Returns (output, BassKernelResults)."""
    in_maps, order, streams = _prepare_in_maps(
        inputs["x"], inputs["edge_index"], inputs["edge_attr"],
        inputs["W1"], inputs["b1"], inputs["W2"], inputs["b2"],
    )
    nc = _get_prog()
    bres = run_bass_kernel_spmd(
        nc, in_maps, list(range(N_CORES)), trace=trace, **spmd_kwargs
    )
    res = bres.results

    out = np.empty((N_EDGES, F_OUT), dtype=np.float32)
    for c in range(N_CORES):
        stream, slen = streams[c]
        core_out = res[c]["out"]  # [128, EPC] fp16 feature-major, col q = stream edge q
        sl_ids = order[c * E_REAL : (c + 1) * E_REAL]
        out[sl_ids[stream]] = core_out[:, :slen].T.astype(np.float32)
    return out, bres


def kernel(x, edge_index, edge_attr, u, batch, W1, b1, W2, b2):
    out, _ = run_spmd(
        {
            "x": x, "edge_index": edge_index, "edge_attr": edge_attr,
            "W1": W1, "b1": b1, "W2": W2, "b2": b2,
        }
    )
    return out


# revision 12
# speedup vs baseline: 1.1975x; 1.0021x over previous
"""Edge-parallel GNN message-passing MLP on 8 TRN2 NeuronCores.

Computation (per edge e): out[e] = relu(concat(x[row[e]], edge_attr[e]) @ W1 + b1) @ W2 + b2

Sharding: edges are SORTED BY ROW on the host, then split evenly across the 8
cores (edge-parallel). Sorting keeps each core's rows in a contiguous band of
~6250 nodes (gather indices fit int16 against the band base) and makes
adjacent edges usually share a row.

Host-side stream construction (per core):
  - edges are packed into PAIRS that share a row (~3% dummy duplicate edges
    inserted at odd-length run boundaries), so one 256 B gather token serves
    two edges: halves gather descriptors and gather HBM traffic
  - x rows are zero-padded to 256 B fp16 tokens [64 feats | 64 zeros]
  - edge_attr is pre-transposed to [64, E] fp16 in stream order
  - the within-tile edge order is chosen so the output DMA writes 4 KiB
    contiguous DRAM per partition (8 consecutive rows), 8x fewer descriptors

Device per 2048-edge tile:
  - dma_gather (non-transpose, 256 B tokens, 1024 pair indices) on SWDGE
    queue t%4 — 4 Q7 core-pairs generate descriptors concurrently
  - PE "transpose" of each 128-pair block against a [128, 256] duplication
    matrix D (D[i,2i]=D[i,2i+1]=1) yields feature-major fp16 x features with
    each pair column already duplicated per edge
  - DVE copies them into the feats tile; edge_attr DMAs into partitions
    64:127; L1 fp16 matmul + fused relu+b1 on ScalarE; L2 per-block matmuls
    (FWL fp16 weight loads); DVE adds b2 -> fp32; 4 KiB-per-partition output
    DMA; host scatters rows back to original edge order.
"""

from contextlib import ExitStack

import numpy as np

import concourse.bacc as bacc_mod
import concourse.bass as bass
import concourse.mybir as mybir
import concourse.tile as tile
from concourse.bass_utils import run_bass_kernel_spmd

N_CORES = 8
N_NODES = 50000
N_EDGES = 800000
F_IN = 64
HIDDEN = 128
F_OUT = 128

E_REAL = N_EDGES // N_CORES  # 100000 edges per core
TILE_E = 2048                # edges per pipeline tile
GRP = 2                      # edges per gather token (same-row groups)
GRPS_T = TILE_E // GRP       # 1024 gather tokens per tile
NT = 51                      # tiles per core (room for ~3% grouping dummies)
EPC = NT * TILE_E            # padded edge-stream length per core
HALF = 1024                  # edges per PSUM-stage half-tile
BAND = 8192                  # max x rows referenced per core (sorted band)

F32 = mybir.dt.float32
F16 = mybir.dt.float16
I16 = mybir.dt.int16

RELU = mybir.ActivationFunctionType.Relu
ADD = mybir.AluOpType.add


def build_program(nt: int = NT):
    epc = nt * TILE_E
    nc = bacc_mod.Bacc("TRN2", num_swdge_queues=4)

    xtok_d = nc.declare_dram_parameter("xtok", [BAND, 2 * F_IN], F16, isOutput=False)
    # pair indices (row - band_base), [16, n/16]-wrapped per tile, replicated
    # x8 across partitions so every SWDGE queue's Q7 pair sees its copy
    idx_d = nc.declare_dram_parameter("idx", [128, nt * (GRPS_T // 16)], I16, isOutput=False)
    eat_d = nc.declare_dram_parameter("eat", [F_IN, epc], F16, isOutput=False)
    ident_d = nc.declare_dram_parameter("ident", [128, 128], F16, isOutput=False)
    w1_d = nc.declare_dram_parameter("w1", [2 * F_IN, HIDDEN], F16, isOutput=False)
    w2_d = nc.declare_dram_parameter("w2", [HIDDEN, F_OUT], F16, isOutput=False)
    b1_d = nc.declare_dram_parameter("b1c", [HIDDEN, 1], F32, isOutput=False)
    b2_d = nc.declare_dram_parameter("b2c", [F_OUT, 1], F32, isOutput=False)
    # feature-major fp16 output: column q = stream edge q; host transposes
    # back and upconverts (fp16 rounding adds ~3e-4 RMS, well within budget)
    out_d = nc.declare_dram_parameter("out", [F_OUT, epc], F16, isOutput=True)

    with tile.TileContext(nc) as tc, ExitStack() as ctx:
        const = ctx.enter_context(tc.tile_pool(name="const", bufs=1))
        xg_p = ctx.enter_context(tc.tile_pool(name="xg", bufs=6))
        feats_p = ctx.enter_context(tc.tile_pool(name="feats", bufs=4))
        h1sb_p = ctx.enter_context(tc.tile_pool(name="h1sb", bufs=4))
        outsb_p = ctx.enter_context(tc.tile_pool(name="outsb", bufs=3))
        xpt_p = ctx.enter_context(tc.tile_pool(name="xpt", bufs=2, space="PSUM"))
        h1ps_p = ctx.enter_context(tc.tile_pool(name="h1ps", bufs=2, space="PSUM"))
        outps_p = ctx.enter_context(tc.tile_pool(name="outps", bufs=2, space="PSUM"))

        # ---- constants (loaded once) ----
        w1_t = const.tile([128, HIDDEN], F16, tag="w1")
        nc.sync.dma_start(out=w1_t, in_=w1_d[:, :])
        w2_t = const.tile([128, F_OUT], F16, tag="w2")
        nc.sync.dma_start(out=w2_t, in_=w2_d[:, :])
        ident_t = const.tile([128, 128], F16, tag="ident")
        nc.sync.dma_start(out=ident_t, in_=ident_d[:, :])
        b1_t = const.tile([128, 1], F32, tag="b1")
        nc.sync.dma_start(out=b1_t, in_=b1_d[:, :])
        b2_t = const.tile([128, 1], F32, tag="b2")
        nc.sync.dma_start(out=b2_t, in_=b2_d[:, :])
        idx_t = const.tile([128, nt * (GRPS_T // 16)], I16, tag="idx")
        nc.sync.dma_start(out=idx_t, in_=idx_d[:, :])

        S = GRPS_T // 16
        for t in range(nt):
            # ---- gather group tokens, group-major [grp%128, grp//128, elem] ----
            xg = xg_p.tile([128, GRPS_T // 128, 2 * F_IN], F16, tag="xg")
            nc.gpsimd.dma_gather(
                xg[:, :, :],
                xtok_d[:, :],
                idx_t[:, t * S : (t + 1) * S],
                GRPS_T,
                GRPS_T,
                2 * F_IN,
                transpose=False,
                single_packet=False,
                queue_num=t % 4,
            )

            # ---- feature-major via PE transposes: xpT[f, grp] ----
            NB = GRPS_T // 128  # 4 transpose blocks
            xpt = xpt_p.tile([F_IN, NB, 128], F16, tag="xpt", space="PSUM")
            for b in range(NB):
                nc.tensor.transpose(
                    out=xpt[:, b, :],
                    in_=xg[:, b, 0:F_IN],
                    identity=ident_t,
                )
            # copy group columns duplicated per edge (stride-0 dup dim), split
            # across DVE and ScalarE: feats col 512*b + 4*k + d = grp 128*b + k
            feats = feats_p.tile([128, TILE_E], F16, tag="feats")
            lo = xpt[:, 0 : NB // 2, :]
            dup_lo = bass.AP(lo.tensor, lo.offset, [*lo.ap, [0, GRP]])
            hi = xpt[:, NB // 2 :, :]
            dup_hi = bass.AP(hi.tensor, hi.offset, [*hi.ap, [0, GRP]])
            nc.vector.tensor_copy(
                out=feats[0:F_IN, 0 : TILE_E // 2].rearrange(
                    "f (b e d) -> f b e d", b=NB // 2, e=128, d=GRP
                ),
                in_=dup_lo,
            )
            nc.scalar.copy(
                out=feats[0:F_IN, TILE_E // 2 : TILE_E].rearrange(
                    "f (b e d) -> f b e d", b=NB // 2, e=128, d=GRP
                ),
                in_=dup_hi,
            )
            nc.sync.dma_start(
                out=feats[F_IN : 2 * F_IN, :],
                in_=eat_d[:, t * TILE_E : (t + 1) * TILE_E],
            )

            # ---- layer 1 (all chunks), then relus, then layer 2 + bias:
            # grouped emission keeps PE matmul streams dense ----
            outsb = outsb_p.tile([128, TILE_E], F16, tag="outsb")
            h1ps_l = []
            for h in range(TILE_E // HALF):
                h1ps = h1ps_p.tile([128, HALF], F32, tag="h1ps", space="PSUM")
                for q in range(HALF // 512):
                    nc.tensor.matmul(
                        out=h1ps[:, q * 512 : (q + 1) * 512],
                        lhsT=w1_t,
                        rhs=feats[:, h * HALF + q * 512 : h * HALF + (q + 1) * 512],
                        start=True,
                        stop=True,
                    )
                h1ps_l.append(h1ps)
            h1sb_l = []
            for h in range(TILE_E // HALF):
                h1sb = h1sb_p.tile([128, HALF], F16, tag="h1sb")
                nc.scalar.activation(
                    out=h1sb, in_=h1ps_l[h], func=RELU, bias=b1_t, scale=1.0
                )
                h1sb_l.append(h1sb)
            for q in range(TILE_E // 512):
                outps = outps_p.tile([128, 512], F32, tag="outps", space="PSUM")
                nc.tensor.matmul(
                    out=outps,
                    lhsT=w2_t,
                    rhs=h1sb_l[q // 2][:, (q % 2) * 512 : (q % 2 + 1) * 512],
                    start=True,
                    stop=True,
                )
                nc.vector.tensor_tensor(
                    out=outsb[:, q * 512 : (q + 1) * 512],
                    in0=outps,
                    in1=b2_t.to_broadcast([128, 512]),
                    op=ADD,
                )
            nc.sync.dma_start(
                out=out_d[:, t * TILE_E : (t + 1) * TILE_E],
                in_=outsb,
            )

    nc.compile()
    return nc


_PROG = None


def _get_prog():
    global _PROG
    if _PROG is None:
        _PROG = build_program(NT)
    return _PROG


def _group_stream(rows_c):
    """Pack sorted rows into same-row groups of GRP edges, duplicating the
    last edge of short runs. Returns local edge indices, one per slot."""
    n = len(rows_c)
    change = np.flatnonzero(np.diff(rows_c)) + 1
    run_starts = np.concatenate([[0], change])
    run_lens = np.diff(np.concatenate([run_starts, [n]]))
    ngrp = (run_lens + GRP - 1) // GRP
    total = int(ngrp.sum())
    grp_run = np.repeat(np.arange(len(run_lens)), ngrp)
    first = np.cumsum(ngrp) - ngrp
    grp_off = np.arange(total) - first[grp_run]
    base = run_starts[grp_run] + GRP * grp_off
    last = run_starts[grp_run] + run_lens[grp_run] - 1
    stream = np.empty((total, GRP), dtype=np.int64)
    for j in range(GRP):
        stream[:, j] = np.minimum(base + j, last)
    return stream.reshape(-1)


def _prepare_in_maps(x, edge_index, edge_attr, W1, b1, W2, b2):
    row = np.ascontiguousarray(np.asarray(edge_index)[0]).astype(np.int64)
    order = np.argsort(row, kind="stable")
    row_s = row[order]
    ea_s = np.asarray(edge_attr, dtype=np.float32)[order]
    x16 = np.asarray(x, dtype=np.float32).astype(np.float16)
    w1_16 = np.ascontiguousarray(np.asarray(W1, dtype=np.float32).astype(np.float16))
    w2_16 = np.ascontiguousarray(np.asarray(W2, dtype=np.float32).astype(np.float16))
    b1c = np.ascontiguousarray(np.asarray(b1, dtype=np.float32).reshape(HIDDEN, 1))
    b2v = np.ascontiguousarray(np.asarray(b2, dtype=np.float32).reshape(F_OUT, 1))

    ident = np.eye(128, dtype=np.float16)

    in_maps = []
    streams = []
    for c in range(N_CORES):
        sl = slice(c * E_REAL, (c + 1) * E_REAL)
        rows_c = row_s[sl]
        r0 = int(rows_c[0])
        band_n = int(rows_c[-1]) - r0 + 1
        assert band_n <= BAND, (c, band_n)

        stream = _group_stream(rows_c)
        assert len(stream) <= EPC, (c, len(stream))
        stream_pad = np.zeros(EPC, dtype=np.int64)
        stream_pad[: len(stream)] = stream
        streams.append((stream, len(stream)))

        grp_rows = (rows_c[stream_pad[0::GRP]] - r0).astype(np.int16)  # [EPC//GRP]
        idx_t = np.ascontiguousarray(
            np.tile(
                grp_rows.reshape(NT, GRPS_T // 16, 16).transpose(0, 2, 1), (1, 8, 1)
            ).transpose(1, 0, 2)
        ).reshape(128, NT * (GRPS_T // 16))

        xb = np.zeros((BAND, 2 * F_IN), dtype=np.float16)
        nb = min(BAND, N_NODES - r0)
        xb[:nb, :F_IN] = x16[r0 : r0 + nb]

        eat = np.ascontiguousarray(ea_s[sl][stream_pad].astype(np.float16).T)

        in_maps.append(
            {
                "xtok": xb,
                "idx": idx_t,
                "eat": eat,
                "ident": ident,
                "w1": w1_16,
                "w2": w2_16,
                "b1c": b1c,
                "b2c": b2v,
            }
        )
    return in_maps, order, streams


def run_spmd(inputs: dict, trace: bool = False, **spmd_kwargs):
    """Run the kernel on all 8 cores. Returns (output, BassKernelResults)."""
    in_maps, order, streams = _prepare_in_maps(
        inputs["x"], inputs["edge_index"], inputs["edge_attr"],
        inputs["W1"], inputs["b1"], inputs["W2"], inputs["b2"],
    )
    nc = _get_prog()
    bres = run_bass_kernel_spmd(
        nc, in_maps, list(range(N_CORES)), trace=trace, **spmd_kwargs
    )
    res = bres.results

    out = np.empty((N_EDGES, F_OUT), dtype=np.float32)
    for c in range(N_CORES):
        stream, slen = streams[c]
        core_out = res[c]["out"]  # [128, EPC] fp16 feature-major, col q = stream edge q
        sl_ids = order[c * E_REAL : (c + 1) * E_REAL]
        out[sl_ids[stream]] = core_out[:, :slen].T.astype(np.float32)
    return out, bres


def kernel(x, edge_index, edge_attr, u, batch, W1, b1, W2, b2):
    out, _ = run_spmd(
        {
            "x": x, "edge_index": edge_index, "edge_attr": edge_attr,
            "W1": W1, "b1": b1, "W2": W2, "b2": b2,
        }
    )
    return out
